# revision 1
# baseline (speedup 1.0000x reference)
"""Trainium2 Bass kernel for nn_Attention_81037442941065.

Dual-attention module (spatial [b,h,n,n] + channel [b,h,d,d]) with
B=2, N=2048, DIM=1024, 16 heads of d=64.

Sharding: 8 cores = (2 batches) x (4 head-groups of 4 heads).
Each core computes its batch/head-group slice end-to-end and produces a
partial (over head groups) output projection; the host sums the 4 group
partials per batch (the "all-reduce after to_out") and adds b_out.

Dtypes: all matmul operands are bf16 (1 cycle/row on the PE; fp32 and
even float32r stream at ~2-4 cycles/row on real TRN2 hardware) with all
accumulation in fp32 PSUM; softmax statistics (sums/reciprocals) in fp32.
Measured end-to-end relative error ~4e-3 vs the fp32 reference.

Per-core layouts (everything "T" is [channels, tokens]):
  z1T, yhT   : [256, 2048]  (transposed projections, head h at rows 64*(h%2)
                             of tile h//2)
  xh_aug     : 16 tiles [128, 260] (natural layout per 128-token chunk;
               per head 65 cols = 64 channels + a ones column so the AV
               matmul also produces the softmax denominators)
  spatial    : S^T = yh @ z1^T computed [keys, queries]; exp on ScalarE
               (scale 1/8 fused, no max subtraction - logits are small);
               AV matmul lhsT=[xh|1] accumulates over key chunks -> rows
               0..63 = unnormalized out1^T, row 64 = sum of exp.
  channel    : [64,64] per head, one PSUM bank each; softmax via
               Exp+accum_out and per-partition reciprocal multiply.
"""

import sys

for _p in ("/opt/trn_rl_repo", "/opt/pypackages"):
    if _p not in sys.path:
        sys.path.insert(0, _p)

import ml_dtypes
import numpy as np
from contextlib import ExitStack

import concourse.bacc as bacc
import concourse.mybir as mybir
import concourse.tile as tile
from concourse.tile import add_dep_helper
from concourse.bass_utils import run_bass_kernel_spmd

F32 = mybir.dt.float32
F32R = mybir.dt.float32r
BF16 = mybir.dt.bfloat16
ATT = mybir.dt.bfloat16   # attention-internal matmul dtype
EXP = mybir.ActivationFunctionType.Exp

B, N, DIM = 2, 2048, 1024
HEADS, DH = 16, 64
G = 4              # head groups == cores per batch
HG = HEADS // G    # heads per group (4)
CIN = HG * DH      # inner channels per core (256)
NCORES = 8
KC = DIM // 128    # contraction chunks for projections (8)
NCH = N // 128     # 128-token chunks (16)
SCALE = DH ** -0.5            # 1/8
CM_SCALE = SCALE / (N / DH)   # 1/256


def _build_program():
    nc = bacc.Bacc(
        "TRN2", target_bir_lowering=False, debug=False, num_devices=NCORES
    )

    # ---- DRAM I/O ----
    xT_d = nc.dram_tensor("xT", [DIM, N], BF16, kind="ExternalInput").ap()
    yT_d = nc.dram_tensor("yT", [DIM, N], BF16, kind="ExternalInput").ap()
    zT_d = nc.dram_tensor("zT", [DIM, N], BF16, kind="ExternalInput").ap()
    wsa1_d = nc.dram_tensor("w_sa1", [DIM, CIN], BF16, kind="ExternalInput").ap()
    wsa2_d = nc.dram_tensor("w_sa2", [DIM, CIN], BF16, kind="ExternalInput").ap()
    wse1_d = nc.dram_tensor("w_se1", [DIM, CIN], BF16, kind="ExternalInput").ap()
    wse2_d = nc.dram_tensor("w_se2", [DIM, CIN], BF16, kind="ExternalInput").ap()
    wout_d = nc.dram_tensor("w_out", [CIN, DIM], ATT, kind="ExternalInput").ap()
    outT_d = nc.dram_tensor("outT", [DIM, N], F32, kind="ExternalOutput").ap()

    with tile.TileContext(nc) as tc, ExitStack() as ctx:
        ppool = ctx.enter_context(tc.tile_pool(name="persist", bufs=1))

        # Persistent projection outputs (live across both scopes).
        z1T = [ppool.tile([128, N], ATT, tag=f"z1T{m}", name=f"z1T{m}")
               for m in range(2)]
        yhT = [ppool.tile([128, N], ATT, tag=f"yhT{m}", name=f"yhT{m}")
               for m in range(2)]
        xh_aug = [ppool.tile([128, HG * (DH + 1)], ATT, tag=f"xa{i}",
                             name=f"xa{i}") for i in range(NCH)]
        secm_sb = [ppool.tile([128, DH], ATT, tag=f"cm{p}", name=f"cm{p}")
                   for p in range(2)]
        rs = [ppool.tile([64, 1], F32, tag=f"rs{h}", name=f"rs{h}")
              for h in range(HG)]
        rcm = [ppool.tile([64, 1], F32, tag=f"rcm{h}", name=f"rcm{h}")
               for h in range(HG)]

        ptpool = ctx.enter_context(tc.tile_pool(name="pt", bufs=4))
        tpool = ctx.enter_context(tc.tile_pool(name="tails", bufs=3))
        opool = ctx.enter_context(tc.tile_pool(name="oout", bufs=4))
        spool = ctx.enter_context(tc.tile_pool(name="spat", bufs=1))
        # w_out as four 64-row slices (base partition 0) matching cat4
        wq = [spool.tile([64, DIM], ATT, tag=f"wq{q}", name=f"wq{q}")
              for q in range(HG)]
        for q in range(HG):
            nc.sync.dma_start(wq[q][:], wout_d[q * 64:(q + 1) * 64, :])
        # cat^T staging: one [64, N] tile per head (this core's inner
        # channels [64h, 64h+64)); the final projection contracts them
        # with matching 64-row slices of w_out
        cat4 = [spool.tile([64, N], ATT, tag=f"cat{h}", name=f"cat{h}")
                for h in range(HG)]

        # ============ Scope 1: all projections + channel-attn logits ======
        with tc.tile_pool(name="proj_in", bufs=1) as ipool, \
             tc.tile_pool(name="psp", bufs=4, space="PSUM") as psp, \
             tc.tile_pool(name="pscm", bufs=1, space="PSUM") as pscm:
            # weights first (small), then x, z, y in consumption order
            wse1_t = [ipool.tile([128, CIN], BF16, tag=f"wse1_{k}",
                                 name=f"wse1_{k}") for k in range(KC)]
            wsa1_t = [ipool.tile([128, CIN], BF16, tag=f"wsa1_{k}",
                                 name=f"wsa1_{k}") for k in range(KC)]
            wse2_t = [ipool.tile([128, CIN], BF16, tag=f"wse2_{k}",
                                 name=f"wse2_{k}") for k in range(KC)]
            wsa2_t = [ipool.tile([128, CIN], BF16, tag=f"wsa2_{k}",
                                 name=f"wsa2_{k}") for k in range(KC)]
            xTt = [ipool.tile([128, N], BF16, tag=f"x{k}", name=f"x{k}")
                   for k in range(KC)]
            zTt = [ipool.tile([128, N], BF16, tag=f"z{k}", name=f"z{k}")
                   for k in range(KC)]
            yTt = [ipool.tile([128, N], BF16, tag=f"y{k}", name=f"y{k}")
                   for k in range(KC)]
            for k in range(KC):
                nc.sync.dma_start(wse1_t[k][:], wse1_d[k * 128:(k + 1) * 128, :])
            for k in range(KC):
                nc.sync.dma_start(xTt[k][:], xT_d[k * 128:(k + 1) * 128, :])
            for k in range(KC):
                nc.sync.dma_start(wsa1_t[k][:], wsa1_d[k * 128:(k + 1) * 128, :])
                nc.sync.dma_start(wse2_t[k][:], wse2_d[k * 128:(k + 1) * 128, :])
            for k in range(KC):
                nc.sync.dma_start(zTt[k][:], zT_d[k * 128:(k + 1) * 128, :])
            for k in range(KC):
                nc.sync.dma_start(wsa2_t[k][:], wsa2_d[k * 128:(k + 1) * 128, :])
            for k in range(KC):
                nc.sync.dma_start(yTt[k][:], yT_d[k * 128:(k + 1) * 128, :])

            cmps = [pscm.tile([64, DH], F32, tag=f"cmp{h}", name=f"cmp{h}")
                    for h in range(HG)]

            # --- xh (natural, augmented with ones) ---
            for i in range(NCH):
                ps = psp.tile([128, 512], F32, tag="pj", name=f"psx{i}")
                for k in range(KC):
                    nc.tensor.matmul(
                        ps[:, 0:CIN],
                        lhsT=xTt[k][:, i * 128:(i + 1) * 128],
                        rhs=wse1_t[k][:],
                        start=(k == 0), stop=(k == KC - 1),
                    )
                src = ps[:, 0:CIN].rearrange("p (h c) -> p h c", c=DH)
                dst = xh_aug[i][:].rearrange("p (h c) -> p h c", c=DH + 1)
                nc.vector.tensor_copy(dst[:, :, 0:DH], src)
                nc.scalar.activation(dst[:, :, DH:DH + 1], src[:, :, 0:1],
                                     mybir.ActivationFunctionType.Copy,
                                     bias=1.0, scale=0.0)

            # --- z1T (transposed projection) ---
            for m in range(2):
                for nb in range(4):
                    ps = psp.tile([128, 512], F32, tag="pj", name=f"psz{m}{nb}")
                    for k in range(KC):
                        nc.tensor.matmul(
                            ps[:],
                            lhsT=wsa1_t[k][:, m * 128:(m + 1) * 128],
                            rhs=zTt[k][:, nb * 512:(nb + 1) * 512],
                            start=(k == 0), stop=(k == KC - 1),
                        )
                    nc.scalar.copy(z1T[m][:, nb * 512:(nb + 1) * 512], ps[:])

            # --- z2 (natural, streamed) + channel-attn logits ---
            for i in range(NCH):
                ps2 = psp.tile([128, 512], F32, tag="pj", name=f"psz2_{i}")
                for k in range(KC):
                    nc.tensor.matmul(
                        ps2[:, 0:CIN],
                        lhsT=zTt[k][:, i * 128:(i + 1) * 128],
                        rhs=wse2_t[k][:],
                        start=(k == 0), stop=(k == KC - 1),
                    )
                z2n = ipool.tile([128, CIN], ATT, tag="z2n", bufs=3,
                                 name=f"z2n{i}")
                nc.scalar.copy(z2n[:], ps2[:, 0:CIN])
                for h in range(HG):
                    nc.tensor.matmul(
                        cmps[h][:],
                        lhsT=xh_aug[i][:, 65 * h:65 * h + DH],
                        rhs=z2n[:, DH * h:DH * (h + 1)],
                        start=(i == 0), stop=(i == NCH - 1),
                    )

            # --- yhT (transposed projection) ---
            for m in range(2):
                for nb in range(4):
                    ps = psp.tile([128, 512], F32, tag="pj", name=f"psy{m}{nb}")
                    for k in range(KC):
                        nc.tensor.matmul(
                            ps[:],
                            lhsT=wsa2_t[k][:, m * 128:(m + 1) * 128],
                            rhs=yTt[k][:, nb * 512:(nb + 1) * 512],
                            start=(k == 0), stop=(k == KC - 1),
                        )
                    nc.scalar.copy(yhT[m][:, nb * 512:(nb + 1) * 512], ps[:])

            # --- channel-attn softmax, DMA'd into pair-packed secm_sb ---
            for h in range(HG):
                p_, off = h // 2, 64 * (h % 2)
                st = ipool.tile([64, DH], ATT, tag="cmstage", bufs=4,
                                name=f"cmstage{h}")
                nc.scalar.activation(st[:], cmps[h][:], EXP,
                                     scale=CM_SCALE,
                                     accum_out=rs[h][0:64, 0:1])
                nc.vector.reciprocal(rcm[h][0:64, 0:1], rs[h][0:64, 0:1])
                nc.vector.tensor_scalar_mul(st[:], st[:], rcm[h][0:64, 0:1])
                nc.sync.dma_start(secm_sb[p_][off:off + 64, :], st[:])

        # ============ Scope 2: out2, spatial attention, final projection ==
        # PSUM: S tag 2x[128,1024] (4 banks) + av 2x[128,512] (2 banks) +
        # aux 2x[128,512] (2 banks) = 8 banks exactly.
        with tc.tile_pool(name="psS", bufs=2, space="PSUM") as psS, \
             tc.tile_pool(name="psAV", bufs=2, space="PSUM") as psAV, \
             tc.tile_pool(name="psaux", bufs=2, space="PSUM") as psaux:

            # Aux matmul stream: out2 + final-projection matmuls, one PE
            # instruction per thunk, drained inside the spatial j-loops so
            # the PE always has ready work while ScalarE runs the exps.
            aux_thunks = []
            final_psf = {}

            def emit_out2(h, nb):
                p_, off = h // 2, 64 * (h % 2)
                pso = psaux.tile([128, 512], F32, tag="aux",
                                 name=f"pso{h}{nb}")
                mm = nc.tensor.matmul(
                    pso[0:64, :],
                    lhsT=secm_sb[p_][off:off + 64, :],
                    rhs=yhT[p_][off:off + 64, nb * 512:(nb + 1) * 512],
                    start=True, stop=True,
                )
                nc.vector.tensor_copy(cat4[h][:, nb * 512:(nb + 1) * 512],
                                      pso[0:64, :])
                return mm

            def emit_final_mm(d, nb, q):
                if q == 0:
                    final_psf[(d, nb)] = psaux.tile(
                        [128, 512], F32, tag="aux", name=f"psf{d}{nb}")
                psf = final_psf[(d, nb)]
                mm = nc.tensor.matmul(
                    psf[:],
                    lhsT=wq[q][:, d * 128:(d + 1) * 128],
                    rhs=cat4[q][:, nb * 512:(nb + 1) * 512],
                    start=(q == 0), stop=(q == HG - 1),
                )
                if q == HG - 1:
                    ob = opool.tile([128, 512], F32, tag="ob",
                                    name=f"ob{d}{nb}")
                    nc.vector.tensor_copy(ob[:], psf[:])
                    nc.sync.dma_start(
                        outT_d[d * 128:(d + 1) * 128,
                               nb * 512:(nb + 1) * 512],
                        ob[:],
                    )
                return mm

            for h in range(HG):
                for nb in range(4):
                    aux_thunks.append(lambda h=h, nb=nb: emit_out2(h, nb))

            def queue_finals(nb, ds=range(8)):
                for d in ds:
                    for q in range(HG):
                        aux_thunks.append(
                            lambda d=d, nb=nb, q=q: emit_final_mm(d, nb, q))

            def drain_aux(k, anchor=None):
                # anchor pins the aux matmul into this drain slot's position
                # in the PE stream - the scheduler's gap-filler otherwise
                # hoists finals into earlier windows where their cat4 inputs
                # are still several microseconds from ready (its cost model
                # underestimates RECIPROCAL ~6x)
                for _ in range(k):
                    if aux_thunks:
                        mm = aux_thunks.pop(0)()
                        if anchor is not None and mm is not None:
                            add_dep_helper(mm.ins, anchor.ins, sync=False,
                                           reason="pin aux to drain slot")

            def make_tail(p_, ib, avs, ptt_last):
                # AV for the last j-pair + normalization tails; emitted at
                # the START of the next iteration so that iteration's S
                # matmuls sit ahead of it in the PE stream (ScalarE usually
                # lags by an exp or two at iteration end).
                icol = ib * 512

                def emit():
                    for hh in range(2):
                        h = 2 * p_ + hh
                        nc.tensor.matmul(
                            avs[hh][0:DH + 1, :],
                            lhsT=xh_aug[NCH - 1][:, 65 * h:65 * h + DH + 1],
                            rhs=ptt_last[:, 512 * hh:512 * hh + 512],
                            start=False, stop=True,
                        )
                    avsbs, rcs, bcs = [], [], []
                    for hh in range(2):
                        avsb = tpool.tile([DH + 1, 512], F32, tag="avsb",
                                          name=f"avsb{p_}{ib}{hh}")
                        nc.vector.tensor_copy(avsb[:], avs[hh][0:DH + 1, :])
                        avsbs.append(avsb)
                    for hh in range(2):
                        rc = tpool.tile([1, 512], F32, tag="rc",
                                        name=f"rc{p_}{ib}{hh}")
                        nc.vector.reciprocal(rc[:], avsbs[hh][DH:DH + 1, :])
                        rcs.append(rc)
                    for hh in range(2):
                        bc = tpool.tile([64, 512], F32, tag="bc",
                                        name=f"bc{p_}{ib}{hh}")
                        nc.gpsimd.partition_broadcast(bc[:], rcs[hh][:])
                        bcs.append(bc)
                    for hh in range(2):
                        h = 2 * p_ + hh
                        tmp = tpool.tile([64, 512], F32, tag="tmp",
                                         name=f"tmp{p_}{ib}{hh}")
                        nc.vector.tensor_mul(tmp[:], avsbs[hh][0:DH, :],
                                             bcs[hh][:])
                        dst = cat4[h][:, icol:icol + 512]
                        nc.vector.tensor_add(dst, tmp[:], dst)
                return emit

            pending_tail = None
            # --- spatial attention: iterations (ib 512-block, pair),
            #     processing key chunks two at a time (j-pairs) ---
            for ib in range(4):
                for p_ in range(2):
                    # nb's cat4 block is complete once BOTH pairs' tails ran;
                    # the second pair's tails execute during (ib+1, p0), so
                    # finals(nb) join the aux queue at (ib+1, p1)
                    if p_ == 1 and ib >= 1:
                        queue_finals(ib - 1)
                    icol = ib * 512
                    avs = [psAV.tile([128, 512], F32, tag="av",
                                     name=f"av{p_}{ib}{q}") for q in range(2)]
                    ptts = [None] * NCH
                    for j in range(NCH):  # key chunks
                        spt = psS.tile([128, 1024], F32, tag="S",
                                       name=f"S{p_}{ib}{j}")
                        s_anchor = None
                        for hh in range(2):
                            off = 64 * hh
                            s_anchor = nc.tensor.matmul(
                                spt[:, 512 * hh:512 * hh + 512],
                                lhsT=yhT[p_][off:off + 64,
                                             j * 128:(j + 1) * 128],
                                rhs=z1T[p_][off:off + 64, icol:icol + 512],
                                start=True, stop=True,
                            )
                        ptt = ptpool.tile([128, 1024], ATT, tag="pt",
                                          name=f"pt{p_}{ib}{j}")
                        nc.scalar.activation(ptt[:], spt[:], EXP, scale=SCALE)
                        ptts[j] = ptt
                        if j == 0 and pending_tail is not None:
                            pending_tail()
                            pending_tail = None
                        drain_aux(1, s_anchor)
                        if j > 0:
                            for hh in range(2):
                                h = 2 * p_ + hh
                                nc.tensor.matmul(
                                    avs[hh][0:DH + 1, :],
                                    lhsT=xh_aug[j - 1][:, 65 * h:65 * h + DH + 1],
                                    rhs=ptts[j - 1][:, 512 * hh:512 * hh + 512],
                                    start=(j == 1), stop=False,
                                )
                    pending_tail = make_tail(p_, ib, avs, ptts[NCH - 1])
            pending_tail()
            queue_finals(3)
            drain_aux(len(aux_thunks))

    nc.compile()
    return nc


_NC_CACHE = {}


def _get_program():
    if "nc" not in _NC_CACHE:
        _NC_CACHE["nc"] = _build_program()
    return _NC_CACHE["nc"]


def _prep_input_maps(x, y, z, w_sa1, w_sa2, w_se1, w_se2, w_out):
    f32 = lambda a: np.ascontiguousarray(np.asarray(a, dtype=np.float32))
    bf16 = lambda a: np.ascontiguousarray(
        np.asarray(a, dtype=np.float32).astype(ml_dtypes.bfloat16))
    maps = []
    for c in range(NCORES):
        b, g = divmod(c, G)
        sl = slice(g * CIN, (g + 1) * CIN)
        maps.append({
            "xT": bf16(np.asarray(x)[b].T),
            "yT": bf16(np.asarray(y)[b].T),
            "zT": bf16(np.asarray(z)[b].T),
            "w_sa1": bf16(np.asarray(w_sa1)[:, sl]),
            "w_sa2": bf16(np.asarray(w_sa2)[:, sl]),
            "w_se1": bf16(np.asarray(w_se1)[:, sl]),
            "w_se2": bf16(np.asarray(w_se2)[:, sl]),
            "w_out": bf16(np.asarray(w_out)[sl, :]),
        })
    return maps


def run(inputs, trace=False, trace_kwargs=None):
    """Run on hardware; returns (full_output, BassKernelResults)."""
    nc = _get_program()
    in_maps = _prep_input_maps(
        inputs["x"], inputs["y"], inputs["z"],
        inputs["w_sa1"], inputs["w_sa2"], inputs["w_se1"], inputs["w_se2"],
        inputs["w_out"],
    )
    res = run_bass_kernel_spmd(
        nc, in_maps, list(range(NCORES)), trace=trace,
        trace_kwargs=trace_kwargs or {},
    )
    out = np.zeros((B, N, DIM), dtype=np.float32)
    for c in range(NCORES):
        b, _g = divmod(c, G)
        out[b] += res.results[c]["outT"].T
    out += np.asarray(inputs["b_out"], dtype=np.float32)
    return out, res


def kernel(**inputs) -> np.ndarray:
    out, _ = run(inputs, trace=False)
    return out



# revision 19
# speedup vs baseline: 1.2452x; 1.2452x over previous
"""Trainium2 Bass kernel for nn_Attention_81037442941065.

Dual-attention module (spatial [b,h,n,n] + channel [b,h,d,d]) with
B=2, N=2048, DIM=1024, 16 heads of d=64.

Sharding: 8 cores = (2 batches) x (4 head-groups of 4 heads).
Each core computes its batch/head-group slice end-to-end and produces a
partial (over head groups) output projection; the host sums the 4 group
partials per batch (the "all-reduce after to_out") and adds b_out.

The wall-clock is bounded by the ScalarE exp of the 16.8M-element S
matrix (~147us at 1 elem/cycle/lane); everything else is structured to
hide behind that stream:
  - DMA order: wsa1/wsa2/wse1, then z/y/x in 1024-column halves so the
    z1T/yhT projections (the spatial-attention inputs) finish first and
    the exp stream starts ~10us into the kernel instead of ~108us.
  - One flat scope. The xh / z2+channel-logit projections are emitted as
    unanchored aux work that the Tile scheduler weaves into PE gaps of
    the early spatial blocks; out2 and the final projection are drained
    (anchored) inside the spatial j-loops, cascaded per query-block.
  - S matmuls are K=64 and rely on Bass's auto tile_position row tiling
    (head hh at partitions 64*hh) so each head-pair runs concurrently on
    the two 64x128 PE tiles. out2 runs as concurrent (64,64) diagonal
    tiles; channel logits as concurrent column tiles.
  - cat is packed two-heads-per-tile ([128, N]) so the final projection
    contracts K=128 in 2 accumulating matmuls instead of 4 K=64 ones.
  - softmax denominators still come free via the ones-column in the AV
    stationary; normalization uses reciprocal_approx_fast (~5x faster
    than the exact DVE reciprocal) and multiplies PSUM-resident AV
    results straight into cat (out2 is added on top afterwards).

Dtypes: all matmul operands bf16, accumulation fp32 PSUM, softmax
statistics fp32. Measured end-to-end relative error ~4e-3 vs the fp32
reference.
"""

import sys

for _p in ("/opt/trn_rl_repo", "/opt/pypackages"):
    if _p not in sys.path:
        sys.path.insert(0, _p)

import ml_dtypes
import numpy as np
from contextlib import ExitStack

import concourse.bacc as bacc
import concourse.mybir as mybir
import concourse.tile as tile
from concourse.tile import add_dep_helper
from concourse.bass_utils import run_bass_kernel_spmd

F32 = mybir.dt.float32
BF16 = mybir.dt.bfloat16
ATT = mybir.dt.bfloat16   # attention-internal matmul dtype
EXP = mybir.ActivationFunctionType.Exp

B, N, DIM = 2, 2048, 1024
HEADS, DH = 16, 64
G = 4              # head groups == cores per batch
HG = HEADS // G    # heads per group (4)
CIN = HG * DH      # inner channels per core (256)
NCORES = 8
KC = DIM // 128    # contraction chunks for projections (8)
NCH = N // 128     # 128-token chunks (16)
SCALE = DH ** -0.5            # 1/8
CM_SCALE = SCALE / (N / DH)   # 1/256
_DEBUG = False                # extra DRAM dumps of intermediates


def _build_program():
    nc = bacc.Bacc(
        "TRN2", target_bir_lowering=False, debug=False, num_devices=NCORES
    )

    # ---- DRAM I/O ----
    xT_d = nc.dram_tensor("xT", [DIM, N], BF16, kind="ExternalInput").ap()
    yT_d = nc.dram_tensor("yT", [DIM, N], BF16, kind="ExternalInput").ap()
    zT_d = nc.dram_tensor("zT", [DIM, N], BF16, kind="ExternalInput").ap()
    wsa1_d = nc.dram_tensor("w_sa1", [DIM, CIN], BF16, kind="ExternalInput").ap()
    wsa2_d = nc.dram_tensor("w_sa2", [DIM, CIN], BF16, kind="ExternalInput").ap()
    wse1_d = nc.dram_tensor("w_se1", [DIM, CIN], BF16, kind="ExternalInput").ap()
    wse2_d = nc.dram_tensor("w_se2", [DIM, CIN], BF16, kind="ExternalInput").ap()
    wout_d = nc.dram_tensor("w_out", [CIN, DIM], ATT, kind="ExternalInput").ap()
    outT_d = nc.dram_tensor("outT", [DIM, N], F32, kind="ExternalOutput").ap()
    if _DEBUG:
        dbg = {
            "dbg_z1q00": nc.dram_tensor("dbg_z1q00", [128, 512], ATT,
                                        kind="ExternalOutput").ap(),
            "dbg_yhk00": nc.dram_tensor("dbg_yhk00", [128, 512], ATT,
                                        kind="ExternalOutput").ap(),
            "dbg_xh0": nc.dram_tensor("dbg_xh0", [128, 260], ATT,
                                      kind="ExternalOutput").ap(),
            "dbg_xh15": nc.dram_tensor("dbg_xh15", [128, 260], ATT,
                                       kind="ExternalOutput").ap(),
            "dbg_cmacc": nc.dram_tensor("dbg_cmacc", [128, 128], F32,
                                        kind="ExternalOutput").ap(),
            "dbg_secm0": nc.dram_tensor("dbg_secm0", [128, 64], ATT,
                                        kind="ExternalOutput").ap(),
            "dbg_secm1": nc.dram_tensor("dbg_secm1", [128, 64], ATT,
                                        kind="ExternalOutput").ap(),
            "dbg_cat0": nc.dram_tensor("dbg_cat0", [128, N], ATT,
                                       kind="ExternalOutput").ap(),
            "dbg_cat1": nc.dram_tensor("dbg_cat1", [128, N], ATT,
                                       kind="ExternalOutput").ap(),
            "dbg_den": nc.dram_tensor("dbg_den", [16, 512], F32,
                                      kind="ExternalOutput").ap(),
            "dbg_rc": nc.dram_tensor("dbg_rc", [16, 512], F32,
                                     kind="ExternalOutput").ap(),
            "dbg_num": nc.dram_tensor("dbg_num", [16, 512], F32,
                                      kind="ExternalOutput").ap(),
        }

    with tile.TileContext(nc) as tc, ExitStack() as ctx:
        ppool = ctx.enter_context(tc.tile_pool(name="persist", bufs=1))

        # z1^T / yh^T in pair-packed layout: tile m holds heads (2m, 2m+1)
        # at partition offsets 0 / 64; split per 512-token block for
        # fine-grained DMA->proj->S pipelining.
        z1q = [[ppool.tile([128, 512], ATT, tag=f"z1q{m}{nb}",
                           name=f"z1q{m}{nb}") for nb in range(4)]
               for m in range(2)]
        yhk = [[ppool.tile([128, 512], ATT, tag=f"yhk{m}{nb}",
                           name=f"yhk{m}{nb}") for nb in range(4)]
               for m in range(2)]
        # xh natural layout per 128-token chunk, 65 cols/head (64 channels
        # + ones column for the softmax denominators)
        xh_aug = [ppool.tile([128, HG * (DH + 1)], ATT, tag=f"xa{i}",
                             name=f"xa{i}") for i in range(NCH)]
        # channel attention: accumulated logits (pair p at cols 64p) and
        # normalized maps (pair-packed rows)
        cmacc = ppool.tile([128, 128], F32, tag="cmacc", name="cmacc")
        secm_sb = [ppool.tile([128, DH], ATT, tag=f"cm{p}", name=f"cm{p}")
                   for p in range(2)]
        rs = [ppool.tile([128, 1], F32, tag=f"rs{p}", name=f"rs{p}")
              for p in range(2)]
        rcm = [ppool.tile([128, 1], F32, tag=f"rcm{p}", name=f"rcm{p}")
               for p in range(2)]
        # cat packed 2 heads per tile: tile p rows 64*hh = head 2p+hh,
        # so the final projection contracts K=128 per tile.
        cat2 = [ppool.tile([128, N], ATT, tag=f"cat{p}", name=f"cat{p}")
                for p in range(2)]
        # w_out as two 128-row slices matching cat2
        w2 = [ppool.tile([128, DIM], ATT, tag=f"w2{p}", name=f"w2{p}")
              for p in range(2)]

        ipool = ctx.enter_context(tc.tile_pool(name="inputs", bufs=1))
        wsa1_t = [ipool.tile([128, CIN], BF16, tag=f"wsa1_{k}",
                             name=f"wsa1_{k}") for k in range(KC)]
        wsa2_t = [ipool.tile([128, CIN], BF16, tag=f"wsa2_{k}",
                             name=f"wsa2_{k}") for k in range(KC)]
        wse1_t = [ipool.tile([128, CIN], BF16, tag=f"wse1_{k}",
                             name=f"wse1_{k}") for k in range(KC)]
        wse2_t = [ipool.tile([128, CIN], BF16, tag=f"wse2_{k}",
                             name=f"wse2_{k}") for k in range(KC)]
        zin = [[ipool.tile([128, 1024], BF16, tag=f"z{k}{hf}",
                           name=f"z{k}{hf}") for hf in range(2)]
               for k in range(KC)]
        yin = [[ipool.tile([128, 1024], BF16, tag=f"y{k}{hf}",
                           name=f"y{k}{hf}") for hf in range(2)]
               for k in range(KC)]
        xin = [[ipool.tile([128, 1024], BF16, tag=f"x{k}{hf}",
                           name=f"x{k}{hf}") for hf in range(2)]
               for k in range(KC)]

        ptpool = ctx.enter_context(tc.tile_pool(name="pt", bufs=6))
        tpool = ctx.enter_context(tc.tile_pool(name="tails", bufs=3))
        zpool = ctx.enter_context(tc.tile_pool(name="z2st", bufs=3))
        opool = ctx.enter_context(tc.tile_pool(name="oout", bufs=4))

        psS = ctx.enter_context(tc.tile_pool(name="psS", bufs=2, space="PSUM"))
        psAV = ctx.enter_context(tc.tile_pool(name="psAV", bufs=2, space="PSUM"))
        psaux = ctx.enter_context(tc.tile_pool(name="psaux", bufs=2,
                                               space="PSUM"))

        # ---- constants: ones columns of xh_aug, zeroed cm accumulator ----
        # ones column FIRST within each head's 65-column group, so the AV
        # softmax denominator lands at PSUM partition 0 (the DVE has no
        # cross-lane path: every vector op below must be lane-aligned)
        for i in range(NCH):
            dst = xh_aug[i][:].rearrange("p (h c) -> p h c", c=DH + 1)
            nc.vector.memset(dst[:, :, 0:1], 1.0)
        nc.vector.memset(cmacc[:], 0.0)

        # ---- DMA, in consumption-priority order ----
        for k in range(KC):
            nc.sync.dma_start(wsa1_t[k][:], wsa1_d[k * 128:(k + 1) * 128, :])
        for k in range(KC):
            nc.sync.dma_start(wsa2_t[k][:], wsa2_d[k * 128:(k + 1) * 128, :])
        for k in range(KC):
            nc.sync.dma_start(wse1_t[k][:], wse1_d[k * 128:(k + 1) * 128, :])
        for k in range(KC):
            nc.sync.dma_start(zin[k][0][:], zT_d[k * 128:(k + 1) * 128, 0:1024])
        for k in range(KC):
            nc.sync.dma_start(yin[k][0][:], yT_d[k * 128:(k + 1) * 128, 0:1024])
        for k in range(KC):
            nc.sync.dma_start(xin[k][0][:], xT_d[k * 128:(k + 1) * 128, 0:1024])
        for k in range(KC):
            nc.sync.dma_start(yin[k][1][:], yT_d[k * 128:(k + 1) * 128, 1024:2048])
        for k in range(KC):
            nc.sync.dma_start(zin[k][1][:], zT_d[k * 128:(k + 1) * 128, 1024:2048])
        for k in range(KC):
            nc.sync.dma_start(xin[k][1][:], xT_d[k * 128:(k + 1) * 128, 1024:2048])
        for k in range(KC):
            nc.sync.dma_start(wse2_t[k][:], wse2_d[k * 128:(k + 1) * 128, :])
        for p in range(2):
            nc.sync.dma_start(w2[p][:], wout_d[p * 128:(p + 1) * 128, :])

        # ---- projection emitters ----
        def emit_pT_block(wt, m, src, dst, nb, label):
            # transposed projection block: dst[128ch, 512tok] for pair m
            ps = psaux.tile([128, 512], F32, tag="aux", name=f"ps{label}")
            for k in range(KC):
                nc.tensor.matmul(
                    ps[:],
                    lhsT=wt[k][:, m * 128:(m + 1) * 128],
                    rhs=src[k][nb // 2][:, (nb % 2) * 512:(nb % 2) * 512 + 512],
                    start=(k == 0), stop=(k == KC - 1),
                )
            nc.scalar.copy(dst[:], ps[:])

        def emit_xh(i):
            # natural-layout xh chunk with strided per-head writes (the
            # ones columns were memset once at kernel start)
            ps = psaux.tile([128, 512], F32, tag="aux", name=f"psx{i}")
            for k in range(KC):
                nc.tensor.matmul(
                    ps[:, 0:CIN],
                    lhsT=xin[k][i // 8][:, (i % 8) * 128:(i % 8) * 128 + 128],
                    rhs=wse1_t[k][:],
                    start=(k == 0), stop=(k == KC - 1),
                )
            src = ps[:, 0:CIN].rearrange("p (h c) -> p h c", c=DH)
            dst = xh_aug[i][:].rearrange("p (h c) -> p h c", c=DH + 1)
            nc.vector.tensor_copy(dst[:, :, 1:DH + 1], src)

        def emit_z2cm(i):
            # z2 chunk + channel-attention logit contribution; cm for head
            # h=2p+hh lands at cmacc[64hh:64hh+64, 64p:64p+64] via
            # concurrent column-tiled matmuls.
            ps2 = psaux.tile([128, 512], F32, tag="aux", name=f"psz2_{i}")
            for k in range(KC):
                nc.tensor.matmul(
                    ps2[:, 0:CIN],
                    lhsT=zin[k][i // 8][:, (i % 8) * 128:(i % 8) * 128 + 128],
                    rhs=wse2_t[k][:],
                    start=(k == 0), stop=(k == KC - 1),
                )
            z2n = zpool.tile([128, CIN], ATT, tag="z2n", name=f"z2n{i}")
            nc.vector.tensor_copy(z2n[:], ps2[:, 0:CIN])
            # matmul start=True marks the full 2KB PSUM partition-row
            # pending-zero, so groups sharing partitions must not share a
            # bank: pair p gets its own bank, heads of a pair split by
            # output partitions (64*hh).
            cmps = [psaux.tile([128, 512], F32, tag="aux", name=f"cmp{i}_{p}")
                    for p in range(2)]
            for h in range(HG):
                p_, hh = h // 2, h % 2
                nc.tensor.matmul(
                    cmps[p_][64 * hh:64 * hh + 64, 0:64],
                    lhsT=xh_aug[i][:, 65 * h + 1:65 * h + 1 + DH],
                    rhs=z2n[:, DH * h:DH * (h + 1)],
                    start=True, stop=True,
                )
            for p_ in range(2):
                dst = cmacc[:, 64 * p_:64 * p_ + 64]
                nc.vector.tensor_add(dst, cmps[p_][:, 0:64], dst)

        def emit_cm_softmax():
            for p in range(2):
                nc.scalar.activation(secm_sb[p][:], cmacc[:, 64 * p:64 * p + 64],
                                     EXP, scale=CM_SCALE,
                                     accum_out=rs[p][:])
                nc.vector.reciprocal_approx_fast(out=rcm[p][:], in_=rs[p][:])
                nc.vector.tensor_scalar_mul(secm_sb[p][:], secm_sb[p][:],
                                            rcm[p][:])

        # ---- pre-spatial emission (unanchored; scheduler fills PE gaps).
        # Priority order matches data arrival + first use.
        emit_pT_block(wsa1_t, 0, zin, z1q[0][0][:], 0, "z1q00")
        emit_pT_block(wsa2_t, 0, yin, yhk[0][0][:], 0, "yhk00")
        emit_pT_block(wsa2_t, 0, yin, yhk[0][1][:], 1, "yhk01")
        for i in range(4):
            emit_xh(i)
        emit_pT_block(wsa2_t, 0, yin, yhk[0][2][:], 2, "yhk02")
        emit_pT_block(wsa2_t, 0, yin, yhk[0][3][:], 3, "yhk03")
        for i in range(4, 8):
            emit_xh(i)
        emit_pT_block(wsa2_t, 1, yin, yhk[1][0][:], 0, "yhk10")
        emit_pT_block(wsa2_t, 1, yin, yhk[1][1][:], 1, "yhk11")
        emit_pT_block(wsa1_t, 1, zin, z1q[1][0][:], 0, "z1q10")
        emit_pT_block(wsa2_t, 1, yin, yhk[1][2][:], 2, "yhk12")
        emit_pT_block(wsa2_t, 1, yin, yhk[1][3][:], 3, "yhk13")
        for i in range(8, NCH):
            emit_xh(i)
        for nb in range(1, 4):
            emit_pT_block(wsa1_t, 0, zin, z1q[0][nb][:], nb, f"z1q0{nb}")
            emit_pT_block(wsa1_t, 1, zin, z1q[1][nb][:], nb, f"z1q1{nb}")
        for i in range(NCH):
            emit_z2cm(i)
        emit_cm_softmax()

        # ---- anchored aux stream: out2 + final projection ----
        aux_thunks = []
        final_psf = {}

        def emit_out2(p_, nb):
            # both heads of pair p_ in one bank via concurrent (64,64)
            # diagonal PE tiles; accumulate into cat2 on top of the
            # normalized out1 written by the tails.
            pso = psaux.tile([128, 512], F32, tag="aux", name=f"pso{p_}{nb}")
            mm = None
            for hh in range(2):
                off = 64 * hh
                mm = nc.tensor.matmul(
                    pso[off:off + 64, :],
                    lhsT=secm_sb[p_][off:off + 64, :],
                    rhs=yhk[p_][nb][off:off + 64, :],
                    start=True, stop=True,
                )
            dst = cat2[p_][:, nb * 512:(nb + 1) * 512]
            nc.vector.tensor_add(dst, pso[:], dst)
            return mm

        def emit_final_mm(d, nb, p):
            if p == 0:
                final_psf[(d, nb)] = psaux.tile(
                    [128, 512], F32, tag="aux", name=f"psf{d}{nb}")
            psf = final_psf[(d, nb)]
            mm = nc.tensor.matmul(
                psf[:],
                lhsT=w2[p][:, d * 128:(d + 1) * 128],
                rhs=cat2[p][:, nb * 512:(nb + 1) * 512],
                start=(p == 0), stop=(p == 1),
            )
            if p == 1:
                ob = opool.tile([128, 512], F32, tag="ob", name=f"ob{d}{nb}")
                nc.vector.tensor_copy(ob[:], psf[:])
                nc.sync.dma_start(
                    outT_d[d * 128:(d + 1) * 128, nb * 512:(nb + 1) * 512],
                    ob[:],
                )
            return mm

        def queue_out2(nb):
            for p_ in range(2):
                aux_thunks.append(lambda p_=p_, nb=nb: emit_out2(p_, nb))

        def queue_finals(nb):
            for d in range(8):
                for p in range(2):
                    aux_thunks.append(
                        lambda d=d, nb=nb, p=p: emit_final_mm(d, nb, p))

        def drain_aux(k, anchor=None):
            # anchor pins the aux matmul into this drain slot's position
            # in the PE stream - the scheduler's gap-filler otherwise
            # hoists finals into windows where their cat2 inputs are not
            # yet written
            for _ in range(k):
                if aux_thunks:
                    mm = aux_thunks.pop(0)()
                    if anchor is not None and mm is not None:
                        add_dep_helper(mm.ins, anchor.ins, sync=False,
                                       reason="pin aux to drain slot")

        def make_tail(p_, ib, avs, ptt_last):
            # AV for the last key chunk + normalization tails; emitted at
            # the START of the next iteration so that iteration's S
            # matmuls sit ahead of it in the PE stream.
            icol = ib * 512

            def emit():
                for hh in range(2):
                    h = 2 * p_ + hh
                    nc.tensor.matmul(
                        avs[hh][0:DH + 1, :],
                        lhsT=xh_aug[NCH - 1][:, 65 * h:65 * h + DH + 1],
                        rhs=ptt_last[:, 512 * hh:512 * hh + 512],
                        start=False, stop=True,
                    )
                # all vector ops below are lane-aligned (base partition 0);
                # the DMA at the end does the cross-partition placement
                # into cat2's 64*hh rows.
                avsbs, rcs, bcs = [], [], []
                for hh in range(2):
                    avsb = tpool.tile([DH + 1, 512], F32, tag="avsb",
                                      name=f"avsb{p_}{ib}{hh}")
                    nc.vector.tensor_copy(avsb[:], avs[hh][0:DH + 1, :])
                    avsbs.append(avsb)
                for hh in range(2):
                    rc = tpool.tile([1, 512], F32, tag="rc",
                                    name=f"rc{p_}{ib}{hh}")
                    nc.vector.reciprocal_approx_fast(
                        out=rc[:], in_=avsbs[hh][0:1, :])
                    rcs.append(rc)
                for hh in range(2):
                    bc = tpool.tile([DH + 1, 512], F32, tag="bc",
                                    name=f"bc{p_}{ib}{hh}")
                    nc.gpsimd.partition_broadcast(bc[:], rcs[hh][:])
                    bcs.append(bc)
                for hh in range(2):
                    # row 0 normalizes the denominator to 1 (unused); rows
                    # 1:65 are the normalized out1 channels
                    tmp = tpool.tile([DH + 1, 512], ATT, tag="tmn",
                                     name=f"tmn{p_}{ib}{hh}")
                    nc.vector.tensor_mul(tmp[:], avsbs[hh][:], bcs[hh][:])
                    nc.sync.dma_start(
                        cat2[p_][64 * hh:64 * hh + 64, icol:icol + 512],
                        tmp[1:DH + 1, :])
                if _DEBUG:
                    for hh in range(2):
                        bi = (ib * 2 + p_) * 2 + hh
                        nc.sync.dma_start(dbg["dbg_den"][bi:bi + 1, :],
                                          avsbs[hh][0:1, :])
                        nc.sync.dma_start(dbg["dbg_rc"][bi:bi + 1, :],
                                          rcs[hh][:])
                        nc.sync.dma_start(dbg["dbg_num"][bi:bi + 1, :],
                                          avsbs[hh][1:2, :])
                return
            return emit

        pending_tail = None
        # ---- spatial attention: blocks (ib = 512-query block, p_ = head
        # pair), key chunks j; S pairs run concurrently via auto row
        # tiling; exp on ScalarE is the pacing stream.
        for ib in range(4):
            for p_ in range(2):
                # aux cascade: out2(nb)/finals(nb) become available one
                # block after cat2's column block nb is fully written
                if (ib, p_) == (1, 1):
                    queue_out2(0)
                    queue_finals(0)
                elif (ib, p_) == (2, 0):
                    queue_out2(1)
                    queue_finals(1)
                elif (ib, p_) == (3, 0):
                    queue_out2(2)
                    queue_finals(2)
                icol = ib * 512
                avs = [psAV.tile([128, 512], F32, tag="av",
                                 name=f"av{p_}{ib}{q}") for q in range(2)]
                ptts = [None] * NCH
                for j in range(NCH):  # key chunks
                    spt = psS.tile([128, 1024], F32, tag="S",
                                   name=f"S{p_}{ib}{j}")
                    s_anchor = None
                    for hh in range(2):
                        off = 64 * hh
                        s_anchor = nc.tensor.matmul(
                            spt[:, 512 * hh:512 * hh + 512],
                            lhsT=yhk[p_][j // 4][off:off + 64,
                                                 (j % 4) * 128:(j % 4) * 128 + 128],
                            rhs=z1q[p_][ib][off:off + 64, :],
                            start=True, stop=True,
                        )
                    ptt = ptpool.tile([128, 1024], ATT, tag="pt",
                                      name=f"pt{p_}{ib}{j}")
                    nc.scalar.activation(ptt[:], spt[:], EXP, scale=SCALE)
                    ptts[j] = ptt
                    if j == 0 and pending_tail is not None:
                        pending_tail()
                        pending_tail = None
                    drain_aux(1, s_anchor)
                    if j > 0:
                        for hh in range(2):
                            h = 2 * p_ + hh
                            nc.tensor.matmul(
                                avs[hh][0:DH + 1, :],
                                lhsT=xh_aug[j - 1][:, 65 * h:65 * h + DH + 1],
                                rhs=ptts[j - 1][:, 512 * hh:512 * hh + 512],
                                start=(j == 1), stop=False,
                            )
                pending_tail = make_tail(p_, ib, avs, ptts[NCH - 1])
        pending_tail()
        queue_out2(3)
        queue_finals(3)
        drain_aux(len(aux_thunks))

        if _DEBUG:
            nc.sync.dma_start(dbg["dbg_z1q00"][:], z1q[0][0][:])
            nc.sync.dma_start(dbg["dbg_yhk00"][:], yhk[0][0][:])
            nc.sync.dma_start(dbg["dbg_xh0"][:], xh_aug[0][:])
            nc.sync.dma_start(dbg["dbg_xh15"][:], xh_aug[15][:])
            nc.sync.dma_start(dbg["dbg_cmacc"][:], cmacc[:])
            nc.sync.dma_start(dbg["dbg_secm0"][:], secm_sb[0][:])
            nc.sync.dma_start(dbg["dbg_secm1"][:], secm_sb[1][:])
            nc.sync.dma_start(dbg["dbg_cat0"][:], cat2[0][:])
            nc.sync.dma_start(dbg["dbg_cat1"][:], cat2[1][:])

    nc.compile()
    return nc


_NC_CACHE = {}


def _get_program():
    if "nc" not in _NC_CACHE:
        _NC_CACHE["nc"] = _build_program()
    return _NC_CACHE["nc"]


def _prep_input_maps(x, y, z, w_sa1, w_sa2, w_se1, w_se2, w_out):
    bf16 = lambda a: np.ascontiguousarray(
        np.asarray(a, dtype=np.float32).astype(ml_dtypes.bfloat16))
    maps = []
    for c in range(NCORES):
        b, g = divmod(c, G)
        sl = slice(g * CIN, (g + 1) * CIN)
        maps.append({
            "xT": bf16(np.asarray(x)[b].T),
            "yT": bf16(np.asarray(y)[b].T),
            "zT": bf16(np.asarray(z)[b].T),
            "w_sa1": bf16(np.asarray(w_sa1)[:, sl]),
            "w_sa2": bf16(np.asarray(w_sa2)[:, sl]),
            "w_se1": bf16(np.asarray(w_se1)[:, sl]),
            "w_se2": bf16(np.asarray(w_se2)[:, sl]),
            "w_out": bf16(np.asarray(w_out)[sl, :]),
        })
    return maps


def run(inputs, trace=False, trace_kwargs=None):
    """Run on hardware; returns (full_output, BassKernelResults)."""
    nc = _get_program()
    in_maps = _prep_input_maps(
        inputs["x"], inputs["y"], inputs["z"],
        inputs["w_sa1"], inputs["w_sa2"], inputs["w_se1"], inputs["w_se2"],
        inputs["w_out"],
    )
    res = run_bass_kernel_spmd(
        nc, in_maps, list(range(NCORES)), trace=trace,
        trace_kwargs=trace_kwargs or {},
    )
    out = np.zeros((B, N, DIM), dtype=np.float32)
    for c in range(NCORES):
        b, _g = divmod(c, G)
        out[b] += res.results[c]["outT"].T
    out += np.asarray(inputs["b_out"], dtype=np.float32)
    return out, res


def kernel(**inputs) -> np.ndarray:
    out, _ = run(inputs, trace=False)
    return out
